# revision 19
# baseline (speedup 1.0000x reference)
"""GateLoop (B=4, N=4096, D=1024) Trainium2 kernel over 8 NeuronCores.

Sharding: data-parallel over the 4 batch elements x 2-way tensor-parallel
split of the D=1024 recurrence channels (the complex diagonal recurrence is
independent per channel). Core c handles batch c//2, channels
[(c%2)*512 : (c%2+1)*512]. Each core computes its projections, runs the
scan over the full sequence for its 512 channels, and produces a partial
y @ wo[ch, :] of shape (1024, 4096) (transposed). The host sums the two
partials per batch and transposes back. No cross-core communication.

Scan formulation (avoids complex arithmetic + overflow): with
a_t = m_t * cis(phi_t), m_t = sigmoid(|a_t|), theta_t = arctan(ai/ar)
in (-pi/2, pi/2) (SIGNED division so the ar<0 half-plane flip folds into
the signed multiplier mt_t = m_t * sign(ar_t)). With Theta_t =
cumsum(theta) the recurrence becomes two independent REAL first-order
scans
    Zr_t = mt_t * Zr_{t-1} + kv_t * cos(Theta_t)
    Zi_t = mt_t * Zi_{t-1} + kv_t * sin(Theta_t)
and Re(S_t) = cos(Theta_t) * Zr_t + sin(Theta_t) * Zi_t, which map onto
the DVE TensorTensorScan instruction (fp32 state, |mt| < 1 so stable).
The Theta scan re-bases each block from the range-reduced thr endpoint
(equivalent mod 2pi, keeps Theta < ~810 in fp32).

Schedule: the elementwise pipeline for block b runs ONE WINDOW LATE
(during block b+1's projections), so every activation-table phase reads
data produced a full window earlier and its gate fires without stalling
the in-order Act queue (head-of-line blocking behind a stalled gated op
was the dominant loss in earlier schedules: it also blocked the PSUM
drains queued behind, starving the PE).
Per 512-token window w (proj of block w on PE):
  [sqrt set]    Sqrt(r2 of w-1)
  [sigmoid set] Arctan(ratio of w-1) + Sigmoid(r of w-1); the DVE
                cumsum/range-reduce chain hangs off the arctans
  [silu set]    Sin(thr|thc of w-1, one wide op per cg) + Silu(g of w-1)
  drains of block w (Square/Sign/Copy live in EVERY set, so the
  scheduler interleaves them freely with the gated phases)
DVE: Theta scans + range reduce (w-1), rec/ratio-clamp drains (w),
mt/wr/wi + Z scans + y1 (w-1). Pool: r2, k*v, q*silu(g), recombination.
Out-projection of block j runs as a 32-matmul burst after proj(j+2).
Projections for (q,g), (k,v), (ar,ai) land in three 2-bank PSUM tiles
per cg so each pair drains with ONE wide Act op; ratio=clamp(ai/ar) is a
single custom DVE op (RT_CLAMP). Weights DMA in consumption order on the
FIFO SP queue. Cross-block scan carries travel as [P,1] column copies.
"""
import math
import os

import numpy as np
import ml_dtypes

B, N, D = 4, 4096, 1024
CH = 512            # channels per core (tensor-parallel half)
NCG = CH // 128     # 4 channel groups of 128 partitions
T = 512             # token block
NBLK = N // T
P = 128
KT = D // P         # contraction tiles
EPS = 1e-5
BF16 = ml_dtypes.bfloat16

TWO_PI = 2 * math.pi
C1 = float(np.float32(6.28125))
C2 = float(np.float32(np.float64(TWO_PI) - 6.28125))
C3 = float(np.float32(np.float64(TWO_PI) - 6.28125
                      - np.float64(np.float32(np.float64(TWO_PI) - 6.28125))))
MAGIC = float(np.float32(1.5 * 2 ** 23))
INV2PI = float(np.float32(1.0 / TWO_PI))
PI = float(np.float32(math.pi))
PIH = float(np.float32(math.pi / 2))
RCLAMP = 1e4

_NC = None
LAST_RESULT = None  # BassKernelResults of the most recent run (for profiling)
_RT_CLAMP = None


def _get_rt_clamp():
    """Register (once) a custom DVE op: out = min(max(in0*in1, s1), s0).

    Fuses the ratio multiply (PSUM ai x SBUF 1/ar) with the arctan-domain
    clamp; 3 uop stages. Registered by appending to concourse.dve_ops.OPS
    with the sha pinned from a local lower() pass.
    """
    global _RT_CLAMP
    if _RT_CLAMP is not None:
        return _RT_CLAMP
    import concourse.dve_ops as dve_ops
    from concourse.dve_ops import DveOp
    from concourse.dve_spec import Spec, Src0, Src1, C0 as SC0, C1 as SC1, \
        lower, minn, maxx, _has_src1
    from concourse.dve_uop import DveOpSpec
    name = "RT_CLAMP_GL"
    if name in dve_ops._SUB_OPCODE_FOR_NAME:
        _RT_CLAMP = next(op for op in dve_ops.OPS if op.name == name)
        return _RT_CLAMP
    spec = Spec(
        body=minn(maxx(Src0 * Src1, SC1), SC0),
        reference=lambda in0, in1, s0, s1, imm2: np.minimum(
            np.maximum(in0.astype(np.float32) * in1, s1), s0
        ).astype(np.float32),
    )
    row = dve_ops._CUSTOM_DVE_ROW_BASE + len(dve_ops.OPS)
    dve_ops._SUB_OPCODE_FOR_NAME[name] = row
    shas = {}
    for ver in ("v3", "v4"):
        uops = lower(spec, ver=ver)
        shas[ver] = DveOpSpec(name=name, opcode=row, uops=uops,
                              rd1_en=_has_src1(spec)).sha(ver)
    op = DveOp(name, spec, subdim=False, uops_sha=shas)
    dve_ops.OPS.append(op)
    dve_ops.CUSTOM_DVE_SPECS[name] = spec
    _RT_CLAMP = op
    return op


def _build():
    from contextlib import ExitStack
    from concourse import bacc
    import concourse.mybir as mybir
    import concourse.tile as tile
    from concourse.mybir import ActivationFunctionType as AF, AluOpType as OP

    fp32 = mybir.dt.float32
    bf = mybir.dt.bfloat16
    rt_clamp = _get_rt_clamp()

    nc = bacc.Bacc(None, target_bir_lowering=False)

    xnT_d = nc.dram_tensor("xnT", [D, N], bf, kind="ExternalInput")
    wnames = ["wq", "wk", "wv", "wg", "war", "wai"]
    w_d = {n: nc.dram_tensor(n, [D, CH], bf, kind="ExternalInput") for n in wnames}
    wo_d = nc.dram_tensor("wo", [CH, D], bf, kind="ExternalInput")
    outT_d = nc.dram_tensor("outT", [D, N], bf, kind="ExternalOutput")

    xnT_t = xnT_d.rearrange("(ko p) n -> p ko n", p=P)
    outT_t = outT_d.rearrange("(mo p) n -> p mo n", p=P)

    with tile.TileContext(nc) as tc, ExitStack() as ctx:
        wpool = ctx.enter_context(tc.tile_pool(name="w", bufs=1))
        xpool = ctx.enter_context(tc.tile_pool(name="x", bufs=2))
        cpool = ctx.enter_context(tc.tile_pool(name="c", bufs=2))   # window-crossing, per block
        kpool = ctx.enter_context(tc.tile_pool(name="k", bufs=2))   # [P,1] scan carries
        scr = ctx.enter_context(tc.tile_pool(name="s", bufs=6))     # fp32 [P,T] scratch
        sc2 = ctx.enter_context(tc.tile_pool(name="s2", bufs=5))    # fp32 [P,2T] scratch
        sbb = ctx.enter_context(tc.tile_pool(name="sb", bufs=12))   # bf16 scratch
        sb2 = ctx.enter_context(tc.tile_pool(name="sb2", bufs=4))   # bf16 [P,2T] scratch
        kvp = ctx.enter_context(tc.tile_pool(name="kv2", bufs=2))   # bf16 [P,2T] k|v drain
        ypool = ctx.enter_context(tc.tile_pool(name="y", bufs=2))
        obp = ctx.enter_context(tc.tile_pool(name="o", bufs=2))
        gpool = ctx.enter_context(tc.tile_pool(name="g", bufs=2))
        pproj = ctx.enter_context(tc.tile_pool(name="pp", bufs=3, space="PSUM"))
        pout = ctx.enter_context(tc.tile_pool(name="po", bufs=2, space="PSUM"))

        # DMA in consumption order on the FIFO SP queue.
        wsb = {}
        wsb["wq"] = wpool.tile([P, KT, CH], bf, tag="w_wq", name="w_wq")
        nc.sync.dma_start(wsb["wq"][:], w_d["wq"].rearrange("(ko p) m -> p ko m", p=P))
        xbs = [None] * NBLK
        xbs[0] = xpool.tile([P, KT, T], bf, tag="xb", name="xb_0")
        nc.sync.dma_start(xbs[0][:], xnT_t[:, :, 0:T])
        for n in ["wg", "wk", "wv", "war", "wai"]:
            t_ = wpool.tile([P, KT, CH], bf, tag=f"w_{n}")
            nc.sync.dma_start(t_[:], w_d[n].rearrange("(ko p) m -> p ko m", p=P))
            wsb[n] = t_
        xbs[1] = xpool.tile([P, KT, T], bf, tag="xb", name="xb_1")
        nc.sync.dma_start(xbs[1][:], xnT_t[:, :, T:2 * T])
        wosb = wpool.tile([P, CH // P, D], bf, tag="w_wo")
        nc.sync.dma_start(wosb[:], wo_d.rearrange("(ko p) m -> p ko m", p=P))

        negmagic = wpool.tile([P, T], fp32, tag="negmagic", name="negmagic")
        nc.gpsimd.memset(negmagic[:], -MAGIC)

        prevThc = [None] * NCG   # [P,1] carry of the reduced Theta endpoint
        prevZr = [None] * NCG    # [P,1] carries of the Z states
        prevZi = [None] * NCG
        ys_all = [None] * NBLK
        gC = 0.0  # gate opening the sqrt set each window

        PAIRS = [("wq", "wg"), ("wk", "wv"), ("war", "wai")]

        def emit_proj(blk):
            xb = xbs[blk]
            PS = [None] * NCG
            for cg in range(NCG):
                cs = slice(cg * P, (cg + 1) * P)
                ps = {}
                for n0, n1 in PAIRS:
                    pt = pproj.tile([P, 2, T], fp32, tag="proj")
                    for h, n in ((0, n0), (1, n1)):
                        for k in range(KT):
                            nc.tensor.matmul(pt[:, h, :], wsb[n][:, k, cs],
                                             xb[:, k, :],
                                             start=(k == 0), stop=(k == KT - 1))
                    ps[n0, n1] = pt
                PS[cg] = ps
            return PS

        def emit_outproj(blk):
            ys = ys_all[blk]
            t0 = blk * T
            for mo in range(D // P):
                pso = pout.tile([P, T], fp32, tag="out")
                for cg in range(NCG):
                    nc.tensor.matmul(pso[:], wosb[:, cg, mo * P:(mo + 1) * P],
                                     ys[cg][:], start=(cg == 0), stop=(cg == NCG - 1))
                ob = obp.tile([P, T], bf, tag="ob")
                # Pool has no PSUM port; alternate the evacuation between
                # Act and DVE explicitly.
                if mo % 4 != 3:
                    nc.scalar.copy(ob[:], pso[:])
                else:
                    nc.vector.tensor_copy(ob[:], pso[:])
                nc.sync.dma_start(outT_t[:, mo, t0:t0 + T], ob[:])

        def emit_stale_phases(pb, pv, gC_in):
            """All table-set phases + DVE/Pool chain for block pb (data in
            pv, produced last window). Returns the gate for next window's
            sqrt set."""
            # sqrt(pb) already ran at the tail of the previous window;
            # its outputs are pv["rr", cg]. Gate the sigmoid set on it.
            gA = gpool.tile([P, 1], fp32, tag="gA", name=f"gA_{pb}")
            nc.vector.tensor_scalar(gA[:], pv["rr", NCG - 1][:, 0:1], 0.0, None,
                                    OP.mult)
            # --- sigmoid set: arctan + sigmoid; DVE chain off the arctans
            sig = [None] * NCG
            tt2s = [None] * NCG
            ths = [None] * NCG
            for cg in range(NCG):
                th = scr.tile([P, T], fp32, tag="scr")
                nc.scalar.activation(th[:], pv["ratio", cg][:], AF.Arctan,
                                     bias=gA[:, 0:1])
                ths[cg] = th
            for cg in range(NCG):
                sg_ = sbb.tile([P, T], bf, tag="sbb", name=f"sig{cg}_{pb}")
                nc.scalar.activation(sg_[:], pv["rr", cg][:], AF.Sigmoid,
                                     bias=gA[:, 0:1])
                sig[cg] = sg_
            for cg in range(NCG):
                Th = scr.tile([P, T], fp32, tag="scr")
                init = 0.0 if pb == 0 else prevThc[cg][:, 0:1]
                nc.vector.tensor_tensor_scan(Th[:], ths[cg][:], ths[cg][:],
                                             init, OP.add, OP.bypass)
                k2 = scr.tile([P, T], fp32, tag="scr")
                nc.vector.affine_then_add(k2[:], Th[:], negmagic[:],
                                          INV2PI, MAGIC)
                tt2 = sc2.tile([P, 2, T], fp32, tag="sc2")
                nc.vector.cody_waite_cascade(tt2[:, 0, :], Th[:], k2[:],
                                             C1, C2, C3)
                nc.vector.add_range_wrap(tt2[:, 1, :], tt2[:, 0, :], PIH, PI,
                                         float(np.float32(TWO_PI)))
                tc_ = kpool.tile([P, 1], fp32, tag=f"thc{cg}",
                                 name=f"thcar{cg}_{pb}")
                nc.vector.tensor_scalar(tc_[:], tt2[:, 0, T - 1:T], 0.0, None,
                                        OP.add)
                prevThc[cg] = tc_
                tt2s[cg] = tt2
            gB = gpool.tile([P, 1], fp32, tag="gB", name=f"gB_{pb}")
            nc.vector.scalar_tensor_tensor(gB[:], sig[NCG - 1][:, 0:1], 0.0,
                                           ths[NCG - 1][:, 0:1],
                                           OP.mult, OP.mult)
            # --- silu set: silus FIRST (forces the silu_and_others load;
            # the sins reuse it), then one wide sin per cg ----------------
            uus = [None] * NCG
            sgs = [None] * NCG
            for cg in range(NCG):
                sl = sbb.tile([P, T], bf, tag="sbb", name=f"sl{cg}_{pb}")
                nc.scalar.activation(sl[:], pv["qg", cg][:, 1, :], AF.Silu,
                                     bias=gB[:, 0:1])
                sgs[cg] = sl
            for cg in range(NCG):
                uu = sb2.tile([P, 2, T], bf, tag="sb2", name=f"uu{cg}_{pb}")
                nc.scalar.activation(uu[:], tt2s[cg][:], AF.Sin, bias=gB[:, 0:1])
                uus[cg] = uu
            gCt = gpool.tile([P, 1], fp32, tag="gC", name=f"gC_{pb}")
            nc.vector.scalar_tensor_tensor(gCt[:], sgs[NCG - 1][:, 0:1], 0.0,
                                           uus[NCG - 1][:, 0, 0:1],
                                           OP.mult, OP.mult)
            # --- DVE/Pool recombination ---------------------------------
            ys = [None] * NCG
            res = [None] * NCG
            qsg = [None] * NCG

            def emit_y1(cg):
                y1 = ypool.tile([P, T], bf, tag=f"y{cg}", name=f"y{cg}_{pb}")
                nc.vector.tensor_tensor(y1[:], qsg[cg][:], res[cg][:], OP.mult)
                ys[cg] = y1

            mts = [None] * NCG
            for cg in range(NCG):
                mt = sbb.tile([P, T], bf, tag="sbb", name=f"mt{cg}_{pb}")
                nc.vector.tensor_tensor(mt[:], sig[cg][:], pv["sgn", cg][:],
                                        OP.mult)
                mts[cg] = mt
            for cg in range(NCG):
                mt = mts[cg]
                qsg[cg] = sbb.tile([P, T], bf, tag="sbb", name=f"qsg{cg}_{pb}")
                nc.gpsimd.tensor_tensor(qsg[cg][:], pv["qg", cg][:, 0, :],
                                        sgs[cg][:], OP.mult)
                wr = sbb.tile([P, T], bf, tag="sbb", name=f"wr{cg}_{pb}")
                nc.gpsimd.tensor_tensor(wr[:], pv["kv", cg][:],
                                        uus[cg][:, 1, :], OP.mult)
                wi = sbb.tile([P, T], bf, tag="sbb", name=f"wi{cg}_{pb}")
                nc.gpsimd.tensor_tensor(wi[:], pv["kv", cg][:],
                                        uus[cg][:, 0, :], OP.mult)
                Zr = sbb.tile([P, T], bf, tag="sbb", name=f"Zr{cg}_{pb}")
                initr = 0.0 if pb == 0 else prevZr[cg][:, 0:1]
                nc.vector.tensor_tensor_scan(Zr[:], mt[:], wr[:], initr,
                                             OP.mult, OP.add)
                Zi = sbb.tile([P, T], bf, tag="sbb", name=f"Zi{cg}_{pb}")
                initi = 0.0 if pb == 0 else prevZi[cg][:, 0:1]
                nc.vector.tensor_tensor_scan(Zi[:], mt[:], wi[:], initi,
                                             OP.mult, OP.add)
                zrc = kpool.tile([P, 1], fp32, tag=f"Zrc{cg}",
                                 name=f"Zrc{cg}_{pb}")
                nc.vector.tensor_scalar(zrc[:], Zr[:, T - 1:T], 0.0, None,
                                        OP.add)
                zic = kpool.tile([P, 1], fp32, tag=f"Zic{cg}",
                                 name=f"Zic{cg}_{pb}")
                nc.vector.tensor_scalar(zic[:], Zi[:, T - 1:T], 0.0, None,
                                        OP.add)
                if cg > 0:
                    emit_y1(cg - 1)
                t1 = sbb.tile([P, T], bf, tag="sbb", name=f"t1{cg}_{pb}")
                nc.gpsimd.tensor_tensor(t1[:], uus[cg][:, 1, :], Zr[:], OP.mult)
                t2 = sbb.tile([P, T], bf, tag="sbb", name=f"t2{cg}_{pb}")
                nc.gpsimd.tensor_tensor(t2[:], uus[cg][:, 0, :], Zi[:], OP.mult)
                re = sbb.tile([P, T], bf, tag="sbb", name=f"re{cg}_{pb}")
                nc.gpsimd.tensor_tensor(re[:], t1[:], t2[:], OP.add)
                res[cg] = re
                prevZr[cg], prevZi[cg] = zrc, zic
            emit_y1(NCG - 1)
            ys_all[pb] = ys
            return gCt

        def emit_drains(b, PS):
            """Fresh PSUM drains for block b: set-free Act ops + DVE
            rec/ratio + Pool kv/r2. Returns the window-crossing tensors."""
            cur = {}
            for cg in range(NCG):
                ps_qg = PS[cg]["wq", "wg"]
                ps_kv = PS[cg]["wk", "wv"]
                ps_a = PS[cg]["war", "wai"]
                qg = cpool.tile([P, 2, T], bf, tag=f"qg{cg}", name=f"qg{cg}_{b}")
                nc.scalar.copy(qg[:], ps_qg[:])
                kvd = kvp.tile([P, 2, T], bf, tag="kvd", name=f"kvd{cg}_{b}")
                nc.scalar.copy(kvd[:], ps_kv[:])
                kv = cpool.tile([P, T], bf, tag=f"kv{cg}", name=f"kv{cg}_{b}")
                nc.gpsimd.tensor_tensor(kv[:], kvd[:, 0, :], kvd[:, 1, :],
                                        OP.mult)
                sq = sc2.tile([P, 2, T], fp32, tag="sc2")
                nc.scalar.square(sq[:], ps_a[:])
                sgn = cpool.tile([P, T], bf, tag=f"sgn{cg}", name=f"sgn{cg}_{b}")
                nc.scalar.sign(sgn[:], ps_a[:, 0, :])
                rec = scr.tile([P, T], fp32, tag="scr")
                nc.vector.reciprocal_approx_fast(rec[:], ps_a[:, 0, :])
                ratio = cpool.tile([P, T], bf, tag=f"ratio{cg}",
                                   name=f"ratio{cg}_{b}")
                nc.vector._custom_dve(rt_clamp, out=ratio[:], in0=ps_a[:, 1, :],
                                      in1=rec[:], s0=RCLAMP, s1=-RCLAMP)
                r2 = cpool.tile([P, T], bf, tag=f"r2{cg}", name=f"r2{cg}_{b}")
                nc.gpsimd.tensor_tensor(r2[:], sq[:, 0, :], sq[:, 1, :], OP.add)
                cur["qg", cg] = qg
                cur["kv", cg] = kv
                cur["sgn", cg] = sgn
                cur["ratio", cg] = ratio
                cur["r2", cg] = r2
            return cur

        def emit_sqrts(b, cur, gC_in):
            # sqrt-set tail of window b: r2(b) is ready mid-window and the
            # silu-set gate fired long before, so this never stalls Act.
            for cg in range(NCG):
                r_ = cpool.tile([P, T], bf, tag=f"rr{cg}", name=f"rr{cg}_{b}")
                nc.scalar.activation(r_[:], cur["r2", cg][:], AF.Sqrt,
                                     bias=gC_in)
                cur["rr", cg] = r_

        prev = None
        for blk in range(NBLK):
            if blk + 2 < NBLK:
                xbs[blk + 2] = xpool.tile([P, KT, T], bf, tag="xb",
                                          name=f"xb_{blk + 2}")
                nc.sync.dma_start(xbs[blk + 2][:],
                                  xnT_t[:, :, (blk + 2) * T:(blk + 3) * T])
            PS = emit_proj(blk)
            if blk >= 2:
                emit_outproj(blk - 2)
            if prev is not None:
                gCt = emit_stale_phases(blk - 1, prev, gC)
                gC = gCt[:, 0:1]
            prev = emit_drains(blk, PS)
            emit_sqrts(blk, prev, gC)

        # tail: the final block's phases, then the last two out-projs
        emit_stale_phases(NBLK - 1, prev, gC)
        emit_outproj(NBLK - 2)
        emit_outproj(NBLK - 1)

    nc.finalize()
    return nc


def _get_nc():
    global _NC
    if _NC is None:
        _NC = _build()
    return _NC


def kernel(**inputs):
    global LAST_RESULT
    from concourse.bass_utils import run_bass_kernel_spmd

    x = np.asarray(inputs["x"], np.float32)
    gamma = np.asarray(inputs["gamma"], np.float32)
    wq = np.asarray(inputs["wq"], np.float32)
    wk = np.asarray(inputs["wk"], np.float32)
    wv = np.asarray(inputs["wv"], np.float32)
    wa = np.asarray(inputs["wa"], np.float32)
    wg = np.asarray(inputs["wg"], np.float32)
    wo = np.asarray(inputs["wo"], np.float32)

    inv = 1.0 / np.sqrt((x * x).sum(-1, keepdims=True) + np.float32(EPS))
    xn = (inv * x * gamma * np.float32(math.sqrt(D))).astype(np.float32)
    xnT = np.ascontiguousarray(xn.transpose(0, 2, 1)).astype(BF16)  # (B, D, N)

    in_maps = []
    for core in range(8):
        b, h = core // 2, core % 2
        ch = slice(h * CH, (h + 1) * CH)
        in_maps.append({
            "xnT": xnT[b],
            "wq": np.ascontiguousarray(wq[:, ch]).astype(BF16),
            "wk": np.ascontiguousarray(wk[:, ch]).astype(BF16),
            "wv": np.ascontiguousarray(wv[:, ch]).astype(BF16),
            "wg": np.ascontiguousarray(wg[:, ch]).astype(BF16),
            "war": np.ascontiguousarray(wa[:, h * CH:(h + 1) * CH]).astype(BF16),
            "wai": np.ascontiguousarray(wa[:, D + h * CH:D + (h + 1) * CH]).astype(BF16),
            "wo": np.ascontiguousarray(wo[ch, :]).astype(BF16),
        })

    nc = _get_nc()
    trace = bool(int(os.environ.get("GATELOOP_TRACE", "0")))
    LAST_RESULT = run_bass_kernel_spmd(
        nc, in_maps, core_ids=list(range(8)), trace=trace,
        trace_cores=list(range(8)) if trace else None,
    )
    res = LAST_RESULT.results

    out = np.empty((B, N, D), np.float32)
    for b in range(B):
        acc = (res[2 * b]["outT"].astype(np.float32)
               + res[2 * b + 1]["outT"].astype(np.float32))   # (D, N)
        out[b] = acc.T
    return out


# revision 20
# speedup vs baseline: 1.1008x; 1.1008x over previous
"""GateLoop (B=4, N=4096, D=1024) Trainium2 kernel over 8 NeuronCores.

Sharding: data-parallel over the 4 batch elements x 2-way tensor-parallel
split of the D=1024 recurrence channels (the complex diagonal recurrence is
independent per channel). Core c handles batch c//2, channels
[(c%2)*512 : (c%2+1)*512]. Each core computes its projections, runs the
scan over the full sequence for its 512 channels, and produces a partial
y @ wo[ch, :] of shape (1024, 4096) (transposed). The host sums the two
partials per batch and transposes back. No cross-core communication.

Scan formulation (avoids complex arithmetic + overflow): with
a_t = m_t * cis(phi_t), m_t = sigmoid(|a_t|), theta_t = arctan(ai/ar)
in (-pi/2, pi/2) (SIGNED division so the ar<0 half-plane flip folds into
the signed multiplier mt_t = m_t * sign(ar_t)). With Theta_t =
cumsum(theta) the recurrence becomes two independent REAL first-order
scans
    Zr_t = mt_t * Zr_{t-1} + kv_t * cos(Theta_t)
    Zi_t = mt_t * Zi_{t-1} + kv_t * sin(Theta_t)
and Re(S_t) = cos(Theta_t) * Zr_t + sin(Theta_t) * Zi_t, which map onto
the DVE TensorTensorScan instruction (fp32 state, |mt| < 1 so stable).
The Theta scan re-bases each block from the range-reduced thr endpoint
(equivalent mod 2pi, keeps Theta < ~810 in fp32).

Schedule: the elementwise pipeline for block b runs ONE WINDOW LATE
(during block b+1's projections), so every activation-table phase reads
data produced a full window earlier and its gate fires without stalling
the in-order Act queue (head-of-line blocking behind a stalled gated op
was the dominant loss in earlier schedules: it also blocked the PSUM
drains queued behind, starving the PE).
Per 512-token window w (proj of block w on PE):
  [sqrt set]    Sqrt(r2 of w-1)
  [sigmoid set] Arctan(ratio of w-1) + Sigmoid(r of w-1); the DVE
                cumsum/range-reduce chain hangs off the arctans
  [silu set]    Sin(thr|thc of w-1, one wide op per cg) + Silu(g of w-1)
  drains of block w (Square/Sign/Copy live in EVERY set, so the
  scheduler interleaves them freely with the gated phases)
DVE: Theta scans + range reduce (w-1), rec/ratio-clamp drains (w),
mt/wr/wi + Z scans + y1 (w-1). Pool: r2, k*v, q*silu(g), recombination.
Out-projection of block j runs as a 32-matmul burst after proj(j+2).
Projections for (q,g), (k,v), (ar,ai) land in three 2-bank PSUM tiles
per cg so each pair drains with ONE wide Act op; ratio=clamp(ai/ar) is a
single custom DVE op (RT_CLAMP). Weights DMA in consumption order on the
FIFO SP queue. Cross-block scan carries travel as [P,1] column copies.
"""
import math
import os

import numpy as np
import ml_dtypes

B, N, D = 4, 4096, 1024
CH = 512            # channels per core (tensor-parallel half)
NCG = CH // 128     # 4 channel groups of 128 partitions
T = 512             # token block
NBLK = N // T
P = 128
KT = D // P         # contraction tiles
EPS = 1e-5
BF16 = ml_dtypes.bfloat16

TWO_PI = 2 * math.pi
C1 = float(np.float32(6.28125))
C2 = float(np.float32(np.float64(TWO_PI) - 6.28125))
C3 = float(np.float32(np.float64(TWO_PI) - 6.28125
                      - np.float64(np.float32(np.float64(TWO_PI) - 6.28125))))
MAGIC = float(np.float32(1.5 * 2 ** 23))
INV2PI = float(np.float32(1.0 / TWO_PI))
PI = float(np.float32(math.pi))
PIH = float(np.float32(math.pi / 2))
RCLAMP = 1e4

_NC = None
LAST_RESULT = None  # BassKernelResults of the most recent run (for profiling)
_RT_CLAMP = None


def _get_rt_clamp():
    """Register (once) a custom DVE op: out = min(max(in0*in1, s1), s0).

    Fuses the ratio multiply (PSUM ai x SBUF 1/ar) with the arctan-domain
    clamp; 3 uop stages. Registered by appending to concourse.dve_ops.OPS
    with the sha pinned from a local lower() pass.
    """
    global _RT_CLAMP
    if _RT_CLAMP is not None:
        return _RT_CLAMP
    import concourse.dve_ops as dve_ops
    from concourse.dve_ops import DveOp
    from concourse.dve_spec import Spec, Src0, Src1, C0 as SC0, C1 as SC1, \
        lower, minn, maxx, _has_src1
    from concourse.dve_uop import DveOpSpec
    name = "RT_CLAMP_GL"
    if name in dve_ops._SUB_OPCODE_FOR_NAME:
        _RT_CLAMP = next(op for op in dve_ops.OPS if op.name == name)
        return _RT_CLAMP
    spec = Spec(
        body=minn(maxx(Src0 * Src1, SC1), SC0),
        reference=lambda in0, in1, s0, s1, imm2: np.minimum(
            np.maximum(in0.astype(np.float32) * in1, s1), s0
        ).astype(np.float32),
    )
    row = dve_ops._CUSTOM_DVE_ROW_BASE + len(dve_ops.OPS)
    dve_ops._SUB_OPCODE_FOR_NAME[name] = row
    shas = {}
    for ver in ("v3", "v4"):
        uops = lower(spec, ver=ver)
        shas[ver] = DveOpSpec(name=name, opcode=row, uops=uops,
                              rd1_en=_has_src1(spec)).sha(ver)
    op = DveOp(name, spec, subdim=False, uops_sha=shas)
    dve_ops.OPS.append(op)
    dve_ops.CUSTOM_DVE_SPECS[name] = spec
    _RT_CLAMP = op
    return op


def _build():
    from contextlib import ExitStack
    from concourse import bacc
    import concourse.mybir as mybir
    import concourse.tile as tile
    from concourse.mybir import ActivationFunctionType as AF, AluOpType as OP

    fp32 = mybir.dt.float32
    bf = mybir.dt.bfloat16
    rt_clamp = _get_rt_clamp()

    nc = bacc.Bacc(None, target_bir_lowering=False)

    xnT_d = nc.dram_tensor("xnT", [D, N], bf, kind="ExternalInput")
    wnames = ["wq", "wk", "wv", "wg", "war", "wai"]
    w_d = {n: nc.dram_tensor(n, [D, CH], bf, kind="ExternalInput") for n in wnames}
    wo_d = nc.dram_tensor("wo", [CH, D], bf, kind="ExternalInput")
    outT_d = nc.dram_tensor("outT", [D, N], bf, kind="ExternalOutput")

    xnT_t = xnT_d.rearrange("(ko p) n -> p ko n", p=P)
    outT_t = outT_d.rearrange("(mo p) n -> p mo n", p=P)

    with tile.TileContext(nc) as tc, ExitStack() as ctx:
        wpool = ctx.enter_context(tc.tile_pool(name="w", bufs=1))
        xpool = ctx.enter_context(tc.tile_pool(name="x", bufs=2))
        cpool = ctx.enter_context(tc.tile_pool(name="c", bufs=2))   # window-crossing, per block
        kpool = ctx.enter_context(tc.tile_pool(name="k", bufs=2))   # [P,1] scan carries
        scr = ctx.enter_context(tc.tile_pool(name="s", bufs=6))     # fp32 [P,T] scratch
        sc2 = ctx.enter_context(tc.tile_pool(name="s2", bufs=5))    # fp32 [P,2T] scratch
        sbb = ctx.enter_context(tc.tile_pool(name="sb", bufs=12))   # bf16 scratch
        sb2 = ctx.enter_context(tc.tile_pool(name="sb2", bufs=4))   # bf16 [P,2T] scratch
        kvp = ctx.enter_context(tc.tile_pool(name="kv2", bufs=2))   # bf16 [P,2T] k|v drain
        ypool = ctx.enter_context(tc.tile_pool(name="y", bufs=2))
        obp = ctx.enter_context(tc.tile_pool(name="o", bufs=2))
        gpool = ctx.enter_context(tc.tile_pool(name="g", bufs=2))
        pproj = ctx.enter_context(tc.tile_pool(name="pp", bufs=3, space="PSUM"))
        pout = ctx.enter_context(tc.tile_pool(name="po", bufs=2, space="PSUM"))

        # DMA in consumption order on the FIFO SP queue.
        wsb = {}
        wsb["wq"] = wpool.tile([P, KT, CH], bf, tag="w_wq", name="w_wq")
        nc.sync.dma_start(wsb["wq"][:], w_d["wq"].rearrange("(ko p) m -> p ko m", p=P))
        xbs = [None] * NBLK
        xbs[0] = xpool.tile([P, KT, T], bf, tag="xb", name="xb_0")
        nc.sync.dma_start(xbs[0][:], xnT_t[:, :, 0:T])
        for n in ["wg", "wk", "wv", "war", "wai"]:
            t_ = wpool.tile([P, KT, CH], bf, tag=f"w_{n}")
            nc.sync.dma_start(t_[:], w_d[n].rearrange("(ko p) m -> p ko m", p=P))
            wsb[n] = t_
        xbs[1] = xpool.tile([P, KT, T], bf, tag="xb", name="xb_1")
        nc.sync.dma_start(xbs[1][:], xnT_t[:, :, T:2 * T])
        wosb = wpool.tile([P, CH // P, D], bf, tag="w_wo")
        nc.sync.dma_start(wosb[:], wo_d.rearrange("(ko p) m -> p ko m", p=P))

        negmagic = wpool.tile([P, T], fp32, tag="negmagic", name="negmagic")
        nc.gpsimd.memset(negmagic[:], -MAGIC)

        prevThc = [None] * NCG   # [P,1] carry of the reduced Theta endpoint
        prevZr = [None] * NCG    # [P,1] carries of the Z states
        prevZi = [None] * NCG
        ys_all = [None] * NBLK
        gC = 0.0  # gate opening the sqrt set each window

        PAIRS = [("wq", "wg"), ("wk", "wv"), ("war", "wai")]

        def emit_proj(blk):
            xb = xbs[blk]
            PS = [None] * NCG
            for cg in range(NCG):
                cs = slice(cg * P, (cg + 1) * P)
                ps = {}
                for n0, n1 in PAIRS:
                    pt = pproj.tile([P, 2, T], fp32, tag="proj")
                    for h, n in ((0, n0), (1, n1)):
                        for k in range(KT):
                            nc.tensor.matmul(pt[:, h, :], wsb[n][:, k, cs],
                                             xb[:, k, :],
                                             start=(k == 0), stop=(k == KT - 1))
                    ps[n0, n1] = pt
                PS[cg] = ps
            return PS

        def emit_outproj(blk):
            ys = ys_all[blk]
            t0 = blk * T
            for mo in range(D // P):
                pso = pout.tile([P, T], fp32, tag="out")
                for cg in range(NCG):
                    nc.tensor.matmul(pso[:], wosb[:, cg, mo * P:(mo + 1) * P],
                                     ys[cg][:], start=(cg == 0), stop=(cg == NCG - 1))
                ob = obp.tile([P, T], bf, tag="ob")
                # Pool has no PSUM port; alternate the evacuation between
                # Act and DVE explicitly.
                if mo % 4 != 3:
                    nc.scalar.copy(ob[:], pso[:])
                else:
                    nc.vector.tensor_copy(ob[:], pso[:])
                nc.sync.dma_start(outT_t[:, mo, t0:t0 + T], ob[:])

        def emit_stale_phases(pb, pv, gC_in):
            """All table-set phases + DVE/Pool chain for block pb (data in
            pv, produced last window). Returns the gate for next window's
            sqrt set."""
            # sqrt(pb) already ran at the tail of the previous window;
            # its outputs are pv["rr", cg]. Gate the sigmoid set on it.
            gA = gpool.tile([P, 1], fp32, tag="gA", name=f"gA_{pb}")
            nc.vector.tensor_scalar(gA[:], pv["rr", NCG - 1][:, 0:1], 0.0, None,
                                    OP.mult)
            # --- sigmoid set: arctan + sigmoid; DVE chain off the arctans
            sig = [None] * NCG
            tt2s = [None] * NCG
            ths = [None] * NCG
            for cg in range(NCG):
                th = scr.tile([P, T], fp32, tag="scr")
                nc.scalar.activation(th[:], pv["ratio", cg][:], AF.Arctan,
                                     bias=gA[:, 0:1])
                ths[cg] = th
            for cg in range(NCG):
                sg_ = sbb.tile([P, T], bf, tag="sbb", name=f"sig{cg}_{pb}")
                nc.scalar.activation(sg_[:], pv["rr", cg][:], AF.Sigmoid,
                                     bias=gA[:, 0:1])
                sig[cg] = sg_
            for cg in range(NCG):
                Th = scr.tile([P, T], fp32, tag="scr")
                init = 0.0 if pb == 0 else prevThc[cg][:, 0:1]
                nc.vector.tensor_tensor_scan(Th[:], ths[cg][:], ths[cg][:],
                                             init, OP.add, OP.bypass)
                k2 = scr.tile([P, T], fp32, tag="scr")
                nc.vector.affine_then_add(k2[:], Th[:], negmagic[:],
                                          INV2PI, MAGIC)
                tt2 = sc2.tile([P, 2, T], fp32, tag="sc2")
                nc.vector.cody_waite_cascade(tt2[:, 0, :], Th[:], k2[:],
                                             C1, C2, C3)
                nc.vector.add_range_wrap(tt2[:, 1, :], tt2[:, 0, :], PIH, PI,
                                         float(np.float32(TWO_PI)))
                tc_ = kpool.tile([P, 1], fp32, tag=f"thc{cg}",
                                 name=f"thcar{cg}_{pb}")
                nc.vector.tensor_scalar(tc_[:], tt2[:, 0, T - 1:T], 0.0, None,
                                        OP.add)
                prevThc[cg] = tc_
                tt2s[cg] = tt2
            gB = gpool.tile([P, 1], fp32, tag="gB", name=f"gB_{pb}")
            nc.vector.scalar_tensor_tensor(gB[:], sig[NCG - 1][:, 0:1], 0.0,
                                           ths[NCG - 1][:, 0:1],
                                           OP.mult, OP.mult)
            # --- silu set: silus FIRST (forces the silu_and_others load;
            # the sins reuse it), then one wide sin per cg ----------------
            uus = [None] * NCG
            sgs = [None] * NCG
            for cg in range(NCG):
                sl = sbb.tile([P, T], bf, tag="sbb", name=f"sl{cg}_{pb}")
                nc.scalar.activation(sl[:], pv["qg", cg][:, 1, :], AF.Silu,
                                     bias=gB[:, 0:1])
                sgs[cg] = sl
            for cg in range(NCG):
                uu = sb2.tile([P, 2, T], bf, tag="sb2", name=f"uu{cg}_{pb}")
                nc.scalar.activation(uu[:], tt2s[cg][:], AF.Sin, bias=gB[:, 0:1])
                uus[cg] = uu
            gCt = gpool.tile([P, 1], fp32, tag="gC", name=f"gC_{pb}")
            nc.vector.scalar_tensor_tensor(gCt[:], sgs[NCG - 1][:, 0:1], 0.0,
                                           uus[NCG - 1][:, 0, 0:1],
                                           OP.mult, OP.mult)
            # --- DVE/Pool recombination ---------------------------------
            ys = [None] * NCG
            res = [None] * NCG
            qsg = [None] * NCG

            def emit_y1(cg):
                y1 = ypool.tile([P, T], bf, tag=f"y{cg}", name=f"y{cg}_{pb}")
                nc.vector.tensor_tensor(y1[:], qsg[cg][:], res[cg][:], OP.mult)
                ys[cg] = y1

            mts = [None] * NCG
            for cg in range(NCG):
                mt = sbb.tile([P, T], bf, tag="sbb", name=f"mt{cg}_{pb}")
                nc.vector.tensor_tensor(mt[:], sig[cg][:], pv["sgn", cg][:],
                                        OP.mult)
                mts[cg] = mt
            for cg in range(NCG):
                mt = mts[cg]
                qsg[cg] = sbb.tile([P, T], bf, tag="sbb", name=f"qsg{cg}_{pb}")
                nc.gpsimd.tensor_tensor(qsg[cg][:], pv["qg", cg][:, 0, :],
                                        sgs[cg][:], OP.mult)
                wr = sbb.tile([P, T], bf, tag="sbb", name=f"wr{cg}_{pb}")
                nc.vector.tensor_tensor(wr[:], pv["kv", cg][:],
                                        uus[cg][:, 1, :], OP.mult)
                wi = sbb.tile([P, T], bf, tag="sbb", name=f"wi{cg}_{pb}")
                nc.vector.tensor_tensor(wi[:], pv["kv", cg][:],
                                        uus[cg][:, 0, :], OP.mult)
                Zr = sbb.tile([P, T], bf, tag="sbb", name=f"Zr{cg}_{pb}")
                initr = 0.0 if pb == 0 else prevZr[cg][:, 0:1]
                nc.vector.tensor_tensor_scan(Zr[:], mt[:], wr[:], initr,
                                             OP.mult, OP.add)
                Zi = sbb.tile([P, T], bf, tag="sbb", name=f"Zi{cg}_{pb}")
                initi = 0.0 if pb == 0 else prevZi[cg][:, 0:1]
                nc.vector.tensor_tensor_scan(Zi[:], mt[:], wi[:], initi,
                                             OP.mult, OP.add)
                zrc = kpool.tile([P, 1], fp32, tag=f"Zrc{cg}",
                                 name=f"Zrc{cg}_{pb}")
                nc.vector.tensor_scalar(zrc[:], Zr[:, T - 1:T], 0.0, None,
                                        OP.add)
                zic = kpool.tile([P, 1], fp32, tag=f"Zic{cg}",
                                 name=f"Zic{cg}_{pb}")
                nc.vector.tensor_scalar(zic[:], Zi[:, T - 1:T], 0.0, None,
                                        OP.add)
                if cg > 0:
                    emit_y1(cg - 1)
                t1 = sbb.tile([P, T], bf, tag="sbb", name=f"t1{cg}_{pb}")
                nc.gpsimd.tensor_tensor(t1[:], uus[cg][:, 1, :], Zr[:], OP.mult)
                t2 = sbb.tile([P, T], bf, tag="sbb", name=f"t2{cg}_{pb}")
                nc.gpsimd.tensor_tensor(t2[:], uus[cg][:, 0, :], Zi[:], OP.mult)
                re = sbb.tile([P, T], bf, tag="sbb", name=f"re{cg}_{pb}")
                nc.gpsimd.tensor_tensor(re[:], t1[:], t2[:], OP.add)
                res[cg] = re
                prevZr[cg], prevZi[cg] = zrc, zic
            emit_y1(NCG - 1)
            ys_all[pb] = ys
            return gCt

        def emit_drains(b, PS):
            """Fresh PSUM drains for block b: set-free Act ops + DVE
            rec/ratio + Pool kv/r2. Returns the window-crossing tensors."""
            cur = {}
            for cg in range(NCG):
                ps_qg = PS[cg]["wq", "wg"]
                ps_kv = PS[cg]["wk", "wv"]
                ps_a = PS[cg]["war", "wai"]
                qg = cpool.tile([P, 2, T], bf, tag=f"qg{cg}", name=f"qg{cg}_{b}")
                nc.scalar.copy(qg[:], ps_qg[:])
                kvd = kvp.tile([P, 2, T], bf, tag="kvd", name=f"kvd{cg}_{b}")
                nc.scalar.copy(kvd[:], ps_kv[:])
                kv = cpool.tile([P, T], bf, tag=f"kv{cg}", name=f"kv{cg}_{b}")
                nc.gpsimd.tensor_tensor(kv[:], kvd[:, 0, :], kvd[:, 1, :],
                                        OP.mult)
                sq = sc2.tile([P, 2, T], fp32, tag="sc2")
                nc.scalar.square(sq[:], ps_a[:])
                sgn = cpool.tile([P, T], bf, tag=f"sgn{cg}", name=f"sgn{cg}_{b}")
                nc.scalar.sign(sgn[:], ps_a[:, 0, :])
                rec = scr.tile([P, T], fp32, tag="scr")
                nc.vector.reciprocal_approx_fast(rec[:], ps_a[:, 0, :])
                ratio = cpool.tile([P, T], bf, tag=f"ratio{cg}",
                                   name=f"ratio{cg}_{b}")
                nc.vector._custom_dve(rt_clamp, out=ratio[:], in0=ps_a[:, 1, :],
                                      in1=rec[:], s0=RCLAMP, s1=-RCLAMP)
                r2 = cpool.tile([P, T], bf, tag=f"r2{cg}", name=f"r2{cg}_{b}")
                nc.gpsimd.tensor_tensor(r2[:], sq[:, 0, :], sq[:, 1, :], OP.add)
                cur["qg", cg] = qg
                cur["kv", cg] = kv
                cur["sgn", cg] = sgn
                cur["ratio", cg] = ratio
                cur["r2", cg] = r2
            return cur

        def emit_sqrts(b, cur, gC_in):
            # sqrt-set tail of window b: r2(b) is ready mid-window and the
            # silu-set gate fired long before, so this never stalls Act.
            for cg in range(NCG):
                r_ = cpool.tile([P, T], bf, tag=f"rr{cg}", name=f"rr{cg}_{b}")
                nc.scalar.activation(r_[:], cur["r2", cg][:], AF.Sqrt,
                                     bias=gC_in)
                cur["rr", cg] = r_

        prev = None
        for blk in range(NBLK):
            if blk + 2 < NBLK:
                xbs[blk + 2] = xpool.tile([P, KT, T], bf, tag="xb",
                                          name=f"xb_{blk + 2}")
                nc.sync.dma_start(xbs[blk + 2][:],
                                  xnT_t[:, :, (blk + 2) * T:(blk + 3) * T])
            PS = emit_proj(blk)
            if blk >= 2:
                emit_outproj(blk - 2)
            if prev is not None:
                gCt = emit_stale_phases(blk - 1, prev, gC)
                gC = gCt[:, 0:1]
            prev = emit_drains(blk, PS)
            emit_sqrts(blk, prev, gC)

        # tail: the final block's phases, then the last two out-projs
        emit_stale_phases(NBLK - 1, prev, gC)
        emit_outproj(NBLK - 2)
        emit_outproj(NBLK - 1)

    nc.finalize()
    return nc


def _get_nc():
    global _NC
    if _NC is None:
        _NC = _build()
    return _NC


def kernel(**inputs):
    global LAST_RESULT
    from concourse.bass_utils import run_bass_kernel_spmd

    x = np.asarray(inputs["x"], np.float32)
    gamma = np.asarray(inputs["gamma"], np.float32)
    wq = np.asarray(inputs["wq"], np.float32)
    wk = np.asarray(inputs["wk"], np.float32)
    wv = np.asarray(inputs["wv"], np.float32)
    wa = np.asarray(inputs["wa"], np.float32)
    wg = np.asarray(inputs["wg"], np.float32)
    wo = np.asarray(inputs["wo"], np.float32)

    inv = 1.0 / np.sqrt((x * x).sum(-1, keepdims=True) + np.float32(EPS))
    xn = (inv * x * gamma * np.float32(math.sqrt(D))).astype(np.float32)
    xnT = np.ascontiguousarray(xn.transpose(0, 2, 1)).astype(BF16)  # (B, D, N)

    in_maps = []
    for core in range(8):
        b, h = core // 2, core % 2
        ch = slice(h * CH, (h + 1) * CH)
        in_maps.append({
            "xnT": xnT[b],
            "wq": np.ascontiguousarray(wq[:, ch]).astype(BF16),
            "wk": np.ascontiguousarray(wk[:, ch]).astype(BF16),
            "wv": np.ascontiguousarray(wv[:, ch]).astype(BF16),
            "wg": np.ascontiguousarray(wg[:, ch]).astype(BF16),
            "war": np.ascontiguousarray(wa[:, h * CH:(h + 1) * CH]).astype(BF16),
            "wai": np.ascontiguousarray(wa[:, D + h * CH:D + (h + 1) * CH]).astype(BF16),
            "wo": np.ascontiguousarray(wo[ch, :]).astype(BF16),
        })

    nc = _get_nc()
    trace = bool(int(os.environ.get("GATELOOP_TRACE", "0")))
    LAST_RESULT = run_bass_kernel_spmd(
        nc, in_maps, core_ids=list(range(8)), trace=trace,
        trace_cores=list(range(8)) if trace else None,
    )
    res = LAST_RESULT.results

    out = np.empty((B, N, D), np.float32)
    for b in range(B):
        acc = (res[2 * b]["outT"].astype(np.float32)
               + res[2 * b + 1]["outT"].astype(np.float32))   # (D, N)
        out[b] = acc.T
    return out
